# revision 3
# baseline (speedup 1.0000x reference)
"""Causal multi-head attention (B=4, S=2048, D=1024, H=16, hd=64) on 8 TRN2 cores.

Sharding: core c handles batch b = c//2 and heads [8*(c%2), 8*(c%2)+8).
Each core computes a partial output y_h @ Wo_rows for its 8 heads over its
batch; the host sums the two partials per batch.

Kernel strategy (per core), v2:
  - x is cast f32->bf16 by a gpsimd casting DMA into DRAM staging, then
    loaded TRANSPOSED into SBUF via the DMA X-bar (no PE transposes).
  - Weights cast-loaded straight to bf16 SBUF via casting DMAs.
  - qT, kT computed in transposed form (lhsT=W-chunk, rhs=xT-chunk); v in
    natural layout with an appended ones column per head (v_aug) so the PV
    matmul also yields softmax denominators.
  - Attention runs per HEAD-PAIR: head A lives on partitions 0-63, head B on
    64-127 of the qT/kT tiles, so the two score matmuls (K=64) land on
    tile_position (0,0) and (64,0) and execute CONCURRENTLY on the PE array.
  - Scores transposed sT[k,q]; exp on ACT straight out of PSUM (logits are
    ~N(0,1): no max subtraction); causal mask = 0/1 multiply on the one
    partially-masked 128-wide block per diagonal k-tile.
  - PV: psum_y[hd+1, q] += vaug^T @ pT, accumulated over k tiles; row hd is
    the softmax denominator.
  - Softmax reciprocal batched per q-chunk: the 8 denominator rows are
    repartitioned [1,4096]->[128,32] with one SBUF->SBUF DMA, DVE
    reciprocal, DMA back, then gpsimd partition_broadcast + DVE multiply
    normalizes yT (no PE broadcast matmuls).
  - Output projection consumes yT directly as lhsT.
  - Dense QKV / output-projection matmuls are interleaved into the
    attention emission stream so the PE never idles (keeps HAM warm):
      phase 0: QKV(0)
      phase 1: ATTN(0) + filler QKV(1)
      phase 2: ATTN(1) + filler QKV(2)
      phase 3: ATTN(2) + filler QKV(3)
      phase 4: ATTN(3) + filler PROJ(0..2)
      phase 5: PROJ(3)
"""

import numpy as np
from contextlib import ExitStack

import concourse.bass as bass
import concourse.tile as tile
from concourse import bacc, mybir
from concourse.bass import ts, ds
from concourse.bass_utils import run_bass_kernel_spmd
from concourse.masks import make_upper_triangular

S = 2048
D = 1024
NH = 8          # heads per core
HD = 64         # head dim
DSH = NH * HD   # 512, per-core shard width
P = 128
F32 = mybir.dt.float32
BF16 = mybir.dt.bfloat16
EXP = mybir.ActivationFunctionType.Exp
SCALE = 1.0 / 8.0  # 1/sqrt(HD)

N_STILES = S // P        # 16
N_QCHUNK = S // 512      # 4
N_DCHUNK = D // P        # 8
N_KCHUNK = DSH // P      # 4
N_PAIR = NH // 2         # 4 head-pairs


def _emit(ctx: ExitStack, tc: tile.TileContext, x_ap, wq_ap, wk_ap, wv_ap, wo_ap, out_ap):
    nc = tc.nc

    const = ctx.enter_context(tc.tile_pool(name="const", bufs=1))
    trimask = const.tile([P, P], BF16, tag="trimask")
    make_upper_triangular(nc, trimask, val=1.0, diag=True)

    # ---- DRAM staging: x cast to bf16 (one tile per s-chunk for precise deps) ----
    dram = ctx.enter_context(tc.tile_pool(name="xbf", bufs=1, space="DRAM"))
    x_bf = [dram.tile([512, D], BF16, tag=f"xbf{sc}", name=f"xbf{sc}") for sc in range(N_QCHUNK)]
    for sc in range(N_QCHUNK):
        nc.gpsimd.dma_start(x_bf[sc][:, :], x_ap[ds(sc * 512, 512), :])

    # ---- weights: casting DMAs straight to bf16 SBUF ----
    wbf = ctx.enter_context(tc.tile_pool(name="wbf", bufs=1))
    wq = [wbf.tile([P, DSH], BF16, tag=f"wq{dc}", name=f"wq{dc}") for dc in range(N_DCHUNK)]
    wk = [wbf.tile([P, DSH], BF16, tag=f"wk{dc}", name=f"wk{dc}") for dc in range(N_DCHUNK)]
    wv = [wbf.tile([P, DSH], BF16, tag=f"wv{dc}", name=f"wv{dc}") for dc in range(N_DCHUNK)]
    for w_list, w_ap in ((wq, wq_ap), (wk, wk_ap), (wv, wv_ap)):
        for dc in range(N_DCHUNK):
            nc.gpsimd.dma_start(w_list[dc][:], w_ap[ts(dc, P), :])
    wo = [wbf.tile([P, D], BF16, tag=f"wo{kc}", name=f"wo{kc}") for kc in range(N_KCHUNK)]
    for kc in range(N_KCHUNK):
        nc.gpsimd.dma_start(wo[kc][:], wo_ap[ts(kc, P), :])

    # ---- xT via X-bar transposed loads ----
    xT_pool = ctx.enter_context(tc.tile_pool(name="xT", bufs=1))
    xT = [xT_pool.tile([P, S], BF16, tag=f"xT{dc}", name=f"xT{dc}") for dc in range(N_DCHUNK)]
    for sc in range(N_QCHUNK):
        for dc in range(N_DCHUNK):
            nc.sync.dma_start(xT[dc][:, ts(sc, 512)], x_bf[sc][:, ts(dc, P)], transpose=True)

    # ---- persistent SBUF tensors ----
    qkT_pool = ctx.enter_context(tc.tile_pool(name="qkT", bufs=1))
    qT = [qkT_pool.tile([P, S], BF16, tag=f"qT{m}", name=f"qT{m}") for m in range(N_KCHUNK)]
    kT = [qkT_pool.tile([P, S], BF16, tag=f"kT{m}", name=f"kT{m}") for m in range(N_KCHUNK)]
    vaug_pool = ctx.enter_context(tc.tile_pool(name="vaug", bufs=1))
    vaug = [vaug_pool.tile([P, NH, HD + 1], BF16, tag=f"v{st}", name=f"v{st}") for st in range(N_STILES)]
    for st in range(N_STILES):
        nc.vector.memset(vaug[st][:, :, HD : HD + 1], 1.0)
    yT_pool = ctx.enter_context(tc.tile_pool(name="yTp", bufs=1))
    yT = [yT_pool.tile([P, S], BF16, tag=f"yT{kc}", name=f"yT{kc}") for kc in range(N_KCHUNK)]

    pT_pool = ctx.enter_context(tc.tile_pool(name="pT", bufs=3))
    ytA_pool = ctx.enter_context(tc.tile_pool(name="ytA", bufs=2))
    r_pool = ctx.enter_context(tc.tile_pool(name="rp", bufs=2))
    rf_pool = ctx.enter_context(tc.tile_pool(name="rf", bufs=3))
    o_pool = ctx.enter_context(tc.tile_pool(name="op", bufs=3))

    psS = ctx.enter_context(tc.tile_pool(name="psS", bufs=1, space="PSUM"))
    psY = ctx.enter_context(tc.tile_pool(name="psY", bufs=2, space="PSUM"))
    dense_ps = ctx.enter_context(tc.tile_pool(name="psD", bufs=2, space="PSUM"))

    # ---------------- unit generators ----------------

    def qkv_units(sc):
        units = []
        for w_list, o_list in ((wq, qT), (wk, kT)):
            for m in range(N_KCHUNK):
                def u(w_list=w_list, o_list=o_list, m=m, sc=sc):
                    pc = dense_ps.tile([P, 512], F32, tag="pc", name=f"qk{sc}{m}{o_list is kT}")
                    for dc in range(N_DCHUNK):
                        nc.tensor.matmul(
                            pc[:],
                            lhsT=w_list[dc][:, ts(m, P)],
                            rhs=xT[dc][:, ts(sc, 512)],
                            start=(dc == 0),
                            stop=(dc == N_DCHUNK - 1),
                        )
                    nc.vector.tensor_copy(o_list[m][:, ts(sc, 512)], pc[:])
                units.append(u)
        for st in range(sc * 4, sc * 4 + 4):
            def u(st=st):
                pc = dense_ps.tile([P, 512], F32, tag="pc", name=f"pv{st}")
                for dc in range(N_DCHUNK):
                    nc.tensor.matmul(
                        pc[:],
                        lhsT=xT[dc][:, ts(st, P)],
                        rhs=wv[dc][:],
                        start=(dc == 0),
                        stop=(dc == N_DCHUNK - 1),
                    )
                nc.vector.tensor_copy(
                    vaug[st][:, :, 0:HD],
                    pc[:].rearrange("p (h d) -> p h d", h=NH),
                )
            units.append(u)
        return units

    # ytAll[qc]: [65, NH, 512] staging of per-head PV output + denominators
    ytAll_tiles = {}

    def attn_units(qc):
        q0 = qc * 512
        n_kt = qc * 4 + 4
        diag0 = qc * 4
        ytAll = ytA_pool.tile([HD + 1, NH, 512], BF16, tag="ytA", name=f"ytA{qc}")
        ytAll_tiles[qc] = ytAll
        units = []
        for t in range(N_PAIR):
            hA, hB = 2 * t, 2 * t + 1
            sides = (
                (hA, kT[t][0:HD, :], qT[t][0:HD, :]),
                (hB, kT[t][HD:P, :], qT[t][HD:P, :]),
            )
            psum_y = {}

            def u_open(t=t, sides=sides, psum_y=psum_y, qc=qc):
                for h, _, _ in sides:
                    psum_y[h] = psY.tile([P, 512], F32, tag="py", name=f"py{qc}{h}")
            # fold open into first pack instead of separate unit
            first = True
            for p0 in range(0, n_kt, 2):
                pack = list(range(p0, min(p0 + 2, n_kt)))

                def u(pack=pack, sides=sides, psum_y=psum_y, first=first,
                      q0=q0, diag0=diag0, n_kt=n_kt, qc=qc, t=t, ytAll=ytAll):
                    if first:
                        for h, _, _ in sides:
                            psum_y[h] = psY.tile([P, 512], F32, tag="py", name=f"py{qc}{h}")
                    strips = {}
                    offs = {}
                    for h, _, _ in sides:
                        strips[h] = psS.tile([P, 1024], F32, tag=f"ps{h % 2}", name=f"ps{qc}{h}{pack[0]}")
                    # scores: interleave A/B so row-tiled pairs run concurrently
                    for idx, kt in enumerate(pack):
                        w = 512 if kt < diag0 else 512 - 128 * (kt - diag0)
                        off = idx * 512
                        qoff = q0 + (512 - w)
                        offs[kt] = (off, w)
                        for h, kT_h, qT_h in sides:
                            nc.tensor.matmul(
                                strips[h][:, ds(off, w)],
                                lhsT=kT_h[:, ts(kt, P)],
                                rhs=qT_h[:, ds(qoff, w)],
                                start=True,
                                stop=True,
                            )
                    # runs for exp (merge contiguous)
                    runs = []
                    for kt in pack:
                        off, w = offs[kt]
                        if runs and runs[-1][1] == off:
                            runs[-1][1] = off + w
                        else:
                            runs.append([off, off + w])
                    pT3s = {}
                    for h, _, _ in sides:
                        pT3 = pT_pool.tile([P, 1024], BF16, tag="pT", name=f"pT{qc}{h}{pack[0]}")
                        pT3s[h] = pT3
                        for r0, r1 in runs:
                            nc.scalar.activation(
                                pT3[:, ds(r0, r1 - r0)], strips[h][:, ds(r0, r1 - r0)], EXP, scale=SCALE
                            )
                        for kt in pack:
                            off, w = offs[kt]
                            if kt >= diag0:
                                nc.vector.tensor_mul(
                                    pT3[:, ds(off, P)], pT3[:, ds(off, P)], trimask[:]
                                )
                    for h, _, _ in sides:
                        for kt in pack:
                            off, w = offs[kt]
                            pcol = 512 - w
                            nc.tensor.matmul(
                                psum_y[h][0 : HD + 1, ds(pcol, w)],
                                lhsT=vaug[kt][:, h, :],
                                rhs=pT3s[h][:, ds(off, w)],
                                start=(kt == 0),
                                stop=(kt == n_kt - 1),
                                skip_group_check=True,
                            )
                    if pack[-1] == n_kt - 1:
                        # last pack: evacuate PV results (+denominator row)
                        for h, _, _ in sides:
                            nc.vector.tensor_copy(ytAll[:, h, :], psum_y[h][0 : HD + 1, :])
                units.append(u)
                first = False
        return units

    def norm_units(qc):
        ytAll = ytAll_tiles[qc]

        def u(qc=qc, ytAll=ytAll):
            # repartition the 8 denominator rows [1, 8*512] -> [128, 32]
            s4 = r_pool.tile([P, 32], BF16, tag="s4")
            nc.sync.dma_start(s4[:], ytAll[HD : HD + 1, :, :])
            r4 = r_pool.tile([P, 32], F32, tag="r4")
            nc.vector.reciprocal(r4[:], s4[:])
            r4b = r_pool.tile([P, 32], BF16, tag="r4b")
            nc.vector.tensor_copy(r4b[:], r4[:])
            rr = r_pool.tile([1, NH * 512], BF16, tag="rr")
            nc.sync.dma_start(rr[:], r4b[:])
            for h in range(NH):
                t, row0 = h // 2, (h % 2) * HD
                rfull = rf_pool.tile([HD, 512], BF16, tag="rfull")
                nc.gpsimd.partition_broadcast(rfull[:], rr[0:1, ds(h * 512, 512)])
                nc.vector.tensor_mul(
                    yT[t][row0 : row0 + HD, ts(qc, 512)], ytAll[0:HD, h, :], rfull[:]
                )
        return [u]

    def proj_units(qc):
        units = []
        for st in range(qc * 4, qc * 4 + 4):
            for ncol in range(2):
                def u(st=st, ncol=ncol):
                    po = dense_ps.tile([P, 512], F32, tag="pc", name=f"po{st}{ncol}")
                    for kc in range(N_KCHUNK):
                        nc.tensor.matmul(
                            po[:],
                            lhsT=yT[kc][:, ts(st, P)],
                            rhs=wo[kc][:, ds(ncol * 512, 512)],
                            start=(kc == 0),
                            stop=(kc == N_KCHUNK - 1),
                        )
                    ot = o_pool.tile([P, 512], F32, tag="o")
                    nc.vector.tensor_copy(ot[:], po[:])
                    nc.sync.dma_start(out_ap[ts(st, P), ds(ncol * 512, 512)], ot[:])
                units.append(u)
        return units

    # ---------------- interleaved emission ----------------

    def run_phase(main, fillers):
        if not main:
            for u in fillers:
                u()
            return
        ratio = len(fillers) / len(main)
        acc, fi = 0.0, 0
        for u in main:
            u()
            acc += ratio
            while acc >= 1.0 and fi < len(fillers):
                fillers[fi]()
                fi += 1
                acc -= 1.0
        while fi < len(fillers):
            fillers[fi]()
            fi += 1

    run_phase(qkv_units(0), [])
    run_phase(attn_units(0) + norm_units(0), qkv_units(1))
    run_phase(attn_units(1) + norm_units(1), qkv_units(2))
    run_phase(attn_units(2) + norm_units(2), qkv_units(3))
    run_phase(attn_units(3) + norm_units(3), proj_units(0) + proj_units(1) + proj_units(2))
    run_phase(proj_units(3), [])


def build_nc():
    nc = bacc.Bacc("TRN2", target_bir_lowering=False, debug=False)
    x_ap = nc.dram_tensor("x", [S, D], F32, kind="ExternalInput").ap()
    wq_ap = nc.dram_tensor("wq", [D, DSH], F32, kind="ExternalInput").ap()
    wk_ap = nc.dram_tensor("wk", [D, DSH], F32, kind="ExternalInput").ap()
    wv_ap = nc.dram_tensor("wv", [D, DSH], F32, kind="ExternalInput").ap()
    wo_ap = nc.dram_tensor("wo", [DSH, D], F32, kind="ExternalInput").ap()
    out_ap = nc.dram_tensor("out", [S, D], F32, kind="ExternalOutput").ap()
    with tile.TileContext(nc) as tc:
        with ExitStack() as ctx:
            _emit(ctx, tc, x_ap, wq_ap, wk_ap, wv_ap, wo_ap, out_ap)
    nc.compile()
    return nc


_NC = None


def _get_nc():
    global _NC
    if _NC is None:
        _NC = build_nc()
    return _NC


def make_in_maps(x, Wqkv, Wo):
    Wq, Wk, Wv = Wqkv[:, 0:D], Wqkv[:, D : 2 * D], Wqkv[:, 2 * D : 3 * D]
    in_maps = []
    for c in range(8):
        b, hh = c // 2, c % 2
        cs = slice(hh * DSH, (hh + 1) * DSH)
        in_maps.append(
            {
                "x": np.ascontiguousarray(x[b], dtype=np.float32),
                "wq": np.ascontiguousarray(Wq[:, cs], dtype=np.float32),
                "wk": np.ascontiguousarray(Wk[:, cs], dtype=np.float32),
                "wv": np.ascontiguousarray(Wv[:, cs], dtype=np.float32),
                "wo": np.ascontiguousarray(Wo[cs, :], dtype=np.float32),
            }
        )
    return in_maps


def kernel(x, Wqkv, Wo, trace=False):
    x = np.asarray(x)
    Wqkv = np.asarray(Wqkv)
    Wo = np.asarray(Wo)
    nc = _get_nc()
    res = run_bass_kernel_spmd(nc, make_in_maps(x, Wqkv, Wo), list(range(8)), trace=trace)
    out = np.empty((4, S, D), np.float32)
    for b in range(4):
        out[b] = res.results[2 * b]["out"] + res.results[2 * b + 1]["out"]
    if trace:
        kernel.last_exec_time_ns = res.exec_time_ns
        kernel.last_results = res
    return out


# revision 10
# speedup vs baseline: 1.1811x; 1.1811x over previous
"""Causal multi-head attention (B=4, S=2048, D=1024, H=16, hd=64) on 8 TRN2 cores.

Sharding: core c handles batch b = c//2 and heads [8*(c%2), 8*(c%2)+8).
Each core computes a partial output y_h @ Wo_rows for its 8 heads over its
batch; the host sums the two partials per batch.

Kernel strategy (per core), v2:
  - x is cast f32->bf16 by a gpsimd casting DMA into DRAM staging, then
    loaded TRANSPOSED into SBUF via the DMA X-bar (no PE transposes).
  - Weights cast-loaded straight to bf16 SBUF via casting DMAs.
  - qT, kT computed in transposed form (lhsT=W-chunk, rhs=xT-chunk); v in
    natural layout with an appended ones column per head (v_aug) so the PV
    matmul also yields softmax denominators.
  - Attention runs per HEAD-PAIR: head A lives on partitions 0-63, head B on
    64-127 of the qT/kT tiles, so the two score matmuls (K=64) land on
    tile_position (0,0) and (64,0) and execute CONCURRENTLY on the PE array.
  - Scores transposed sT[k,q]; exp on ACT straight out of PSUM (logits are
    ~N(0,1): no max subtraction); causal mask = 0/1 multiply on the one
    partially-masked 128-wide block per diagonal k-tile.
  - PV: psum_y[hd+1, q] += vaug^T @ pT, accumulated over k tiles; row hd is
    the softmax denominator.
  - Softmax reciprocal batched per q-chunk: the 8 denominator rows are
    repartitioned [1,4096]->[128,32] with one SBUF->SBUF DMA, DVE
    reciprocal, DMA back, then gpsimd partition_broadcast + DVE multiply
    normalizes yT (no PE broadcast matmuls).
  - Output projection consumes yT directly as lhsT.
  - Dense QKV / output-projection matmuls are interleaved into the
    attention emission stream so the PE never idles (keeps HAM warm):
      phase 0: QKV(0)
      phase 1: ATTN(0) + filler QKV(1)
      phase 2: ATTN(1) + filler QKV(2)
      phase 3: ATTN(2) + filler QKV(3)
      phase 4: ATTN(3) + filler PROJ(0..2)
      phase 5: PROJ(3)
"""

import numpy as np
from contextlib import ExitStack

import concourse.bass as bass
import concourse.tile as tile
from concourse import bacc, mybir
from concourse.bass import ts, ds
from concourse.bass_utils import run_bass_kernel_spmd
from concourse.masks import make_identity, make_upper_triangular

S = 2048
D = 1024
NH = 8          # heads per core
HD = 64         # head dim
DSH = NH * HD   # 512, per-core shard width
P = 128
F32 = mybir.dt.float32
BF16 = mybir.dt.bfloat16
EXP = mybir.ActivationFunctionType.Exp
SCALE = 1.0 / 8.0  # 1/sqrt(HD)

N_STILES = S // P        # 16
N_QCHUNK = S // 512      # 4
N_DCHUNK = D // P        # 8
N_KCHUNK = DSH // P      # 4
N_PAIR = NH // 2         # 4 head-pairs


def _emit(ctx: ExitStack, tc: tile.TileContext, x_ap, wq_ap, wk_ap, wv_ap, wo_ap, out_ap):
    nc = tc.nc

    const = ctx.enter_context(tc.tile_pool(name="const", bufs=1))
    trimask = const.tile([P, P], BF16, tag="trimask")
    make_upper_triangular(nc, trimask, val=1.0, diag=True)
    ident = const.tile([P, P], F32, tag="ident")
    make_identity(nc, ident)
    ident_bf = const.tile([P, P], BF16, tag="ident_bf")
    nc.vector.tensor_copy(ident_bf[:], ident[:])

    # ---- weights: efficient f32 HWDGE loads, cast to bf16 on GpSimd ----
    ld_pool = ctx.enter_context(tc.tile_pool(name="ld", bufs=2))
    wbf = ctx.enter_context(tc.tile_pool(name="wbf", bufs=1))
    wq = [wbf.tile([P, DSH], BF16, tag=f"wq{dc}", name=f"wq{dc}") for dc in range(N_DCHUNK)]
    wk = [wbf.tile([P, DSH], BF16, tag=f"wk{dc}", name=f"wk{dc}") for dc in range(N_DCHUNK)]
    wv = [wbf.tile([P, DSH], BF16, tag=f"wv{dc}", name=f"wv{dc}") for dc in range(N_DCHUNK)]
    wo = [wbf.tile([P, D], BF16, tag=f"wo{kc}", name=f"wo{kc}") for kc in range(N_KCHUNK)]

    def emit_weight_loads():
        for w_list, w_ap in ((wq, wq_ap), (wk, wk_ap), (wv, wv_ap)):
            for dc in range(N_DCHUNK):
                wt = ld_pool.tile([P, DSH], F32, tag="w")
                nc.sync.dma_start(wt[:], w_ap[ts(dc, P), :])
                nc.gpsimd.tensor_copy(w_list[dc][:], wt[:])
        for kc in range(N_KCHUNK):
            wt = ld_pool.tile([P, D], F32, tag="w")
            nc.sync.dma_start(wt[:], wo_ap[ts(kc, P), :])
            nc.gpsimd.tensor_copy(wo[kc][:], wt[:])

    # ---- x: f32 loads + gpsimd cast; PE transposes emitted per s-chunk ----
    xT_pool = ctx.enter_context(tc.tile_pool(name="xT", bufs=1))
    xT = [xT_pool.tile([P, S], BF16, tag=f"xT{dc}", name=f"xT{dc}") for dc in range(N_DCHUNK)]
    xb_pool = ctx.enter_context(tc.tile_pool(name="xb", bufs=4))

    def emit_x_loads(sc):
        xbs = []
        for st in range(sc * 4, sc * 4 + 4):
            xt = ld_pool.tile([P, D], F32, tag="x", bufs=4)
            nc.sync.dma_start(xt[:], x_ap[ts(st, P), :])
            xb = xb_pool.tile([P, D], BF16, tag="xb")
            nc.gpsimd.tensor_copy(xb[:], xt[:])
            xbs.append(xb)
        return xbs

    def transpose_units(sc, xbs):
        units = []
        for dc in range(N_DCHUNK):
            def u(dc=dc, sc=sc, xbs=xbs):
                pt = dense_ps.tile([P, 512], BF16, tag="pc", name=f"pt{sc}{dc}")
                for j in range(4):
                    nc.tensor.transpose(pt[:, ts(j, P)], xbs[j][:, ts(dc, P)], ident_bf)
                nc.vector.tensor_copy(xT[dc][:, ds(sc * 512 , 512)], pt[:])
            units.append(u)
        return units

    # ---- persistent SBUF tensors ----
    qkT_pool = ctx.enter_context(tc.tile_pool(name="qkT", bufs=1))
    qT = [qkT_pool.tile([P, S], BF16, tag=f"qT{m}", name=f"qT{m}") for m in range(N_KCHUNK)]
    kT = [qkT_pool.tile([P, S], BF16, tag=f"kT{m}", name=f"kT{m}") for m in range(N_KCHUNK)]
    vaug_pool = ctx.enter_context(tc.tile_pool(name="vaug", bufs=1))
    vaug = [vaug_pool.tile([P, NH, HD + 1], BF16, tag=f"v{st}", name=f"v{st}") for st in range(N_STILES)]
    for st in range(N_STILES):
        nc.vector.memset(vaug[st][:, :, HD : HD + 1], 1.0)
    yT_pool = ctx.enter_context(tc.tile_pool(name="yTp", bufs=1))
    yT = [yT_pool.tile([P, S], BF16, tag=f"yT{kc}", name=f"yT{kc}") for kc in range(N_KCHUNK)]

    pT_pool = ctx.enter_context(tc.tile_pool(name="pT", bufs=3))
    ytA_pool = ctx.enter_context(tc.tile_pool(name="ytA", bufs=2))
    r_pool = ctx.enter_context(tc.tile_pool(name="rp", bufs=2))
    rr_pool = ctx.enter_context(tc.tile_pool(name="rrp", bufs=1))
    rf_pool = ctx.enter_context(tc.tile_pool(name="rf", bufs=3))
    o_pool = ctx.enter_context(tc.tile_pool(name="op", bufs=2))

    psS = ctx.enter_context(tc.tile_pool(name="psS", bufs=1, space="PSUM"))
    psY = ctx.enter_context(tc.tile_pool(name="psY", bufs=2, space="PSUM"))
    dense_ps = ctx.enter_context(tc.tile_pool(name="psD", bufs=2, space="PSUM"))

    # ---------------- unit generators ----------------

    def qkv_units(sc):
        units = []
        for w_list, o_list in ((wq, qT), (wk, kT)):
            for m in range(N_KCHUNK):
                def u(w_list=w_list, o_list=o_list, m=m, sc=sc):
                    pc = dense_ps.tile([P, 512], F32, tag="pc", name=f"qk{sc}{m}{o_list is kT}")
                    for dc in range(N_DCHUNK):
                        nc.tensor.matmul(
                            pc[:],
                            lhsT=w_list[dc][:, ts(m, P)],
                            rhs=xT[dc][:, ts(sc, 512)],
                            start=(dc == 0),
                            stop=(dc == N_DCHUNK - 1),
                        )
                    nc.vector.tensor_copy(o_list[m][:, ts(sc, 512)], pc[:])
                units.append(u)
        for st in range(sc * 4, sc * 4 + 4):
            def u(st=st):
                pc = dense_ps.tile([P, 512], F32, tag="pc", name=f"pv{st}")
                for dc in range(N_DCHUNK):
                    nc.tensor.matmul(
                        pc[:],
                        lhsT=xT[dc][:, ts(st, P)],
                        rhs=wv[dc][:],
                        start=(dc == 0),
                        stop=(dc == N_DCHUNK - 1),
                    )
                nc.vector.tensor_copy(
                    vaug[st][:, :, 0:HD],
                    pc[:].rearrange("p (h d) -> p h d", h=NH),
                )
            units.append(u)
        return units

    # ytAll[qc]: [65, NH, 512] staging of per-head PV output + denominators
    ytAll_tiles = {}

    def attn_units(qc):
        q0 = qc * 512
        n_kt = qc * 4 + 4
        diag0 = qc * 4
        ytAll = ytA_pool.tile([HD + 1, NH, 512], BF16, tag="ytA", name=f"ytA{qc}")
        ytAll_tiles[qc] = ytAll
        units = []
        for t in range(N_PAIR):
            hA, hB = 2 * t, 2 * t + 1
            sides = (
                (hA, kT[t][0:HD, :], qT[t][0:HD, :]),
                (hB, kT[t][HD:P, :], qT[t][HD:P, :]),
            )
            psum_y = {}

            def u_open(t=t, sides=sides, psum_y=psum_y, qc=qc):
                for h, _, _ in sides:
                    psum_y[h] = psY.tile([P, 512], F32, tag="py", name=f"py{qc}{h}")
            # fold open into first pack instead of separate unit
            first = True
            for p0 in range(0, n_kt, 2):
                pack = list(range(p0, min(p0 + 2, n_kt)))

                def u(pack=pack, sides=sides, psum_y=psum_y, first=first,
                      q0=q0, diag0=diag0, n_kt=n_kt, qc=qc, t=t, ytAll=ytAll):
                    if first:
                        for h, _, _ in sides:
                            psum_y[h] = psY.tile([P, 512], F32, tag="py", name=f"py{qc}{h}")
                    strips = {}
                    offs = {}
                    for h, _, _ in sides:
                        strips[h] = psS.tile([P, 1024], F32, tag=f"ps{h % 2}", name=f"ps{qc}{h}{pack[0]}")
                    # scores: interleave A/B so row-tiled pairs run concurrently
                    for idx, kt in enumerate(pack):
                        w = 512 if kt < diag0 else 512 - 128 * (kt - diag0)
                        off = idx * 512
                        qoff = q0 + (512 - w)
                        offs[kt] = (off, w)
                        for h, kT_h, qT_h in sides:
                            nc.tensor.matmul(
                                strips[h][:, ds(off, w)],
                                lhsT=kT_h[:, ts(kt, P)],
                                rhs=qT_h[:, ds(qoff, w)],
                                start=True,
                                stop=True,
                            )
                    # runs for exp (merge contiguous)
                    runs = []
                    for kt in pack:
                        off, w = offs[kt]
                        if runs and runs[-1][1] == off:
                            runs[-1][1] = off + w
                        else:
                            runs.append([off, off + w])
                    pT3s = {}
                    for h, _, _ in sides:
                        pT3 = pT_pool.tile([P, 1024], BF16, tag="pT", name=f"pT{qc}{h}{pack[0]}")
                        pT3s[h] = pT3
                        for r0, r1 in runs:
                            nc.scalar.activation(
                                pT3[:, ds(r0, r1 - r0)], strips[h][:, ds(r0, r1 - r0)], EXP, scale=SCALE
                            )
                        for kt in pack:
                            off, w = offs[kt]
                            if kt >= diag0:
                                nc.vector.tensor_mul(
                                    pT3[:, ds(off, P)], pT3[:, ds(off, P)], trimask[:]
                                )
                    for h, _, _ in sides:
                        for kt in pack:
                            off, w = offs[kt]
                            pcol = 512 - w
                            nc.tensor.matmul(
                                psum_y[h][0 : HD + 1, ds(pcol, w)],
                                lhsT=vaug[kt][:, h, :],
                                rhs=pT3s[h][:, ds(off, w)],
                                start=(kt == 0),
                                stop=(kt == n_kt - 1),
                                skip_group_check=True,
                            )
                    if pack[-1] == n_kt - 1:
                        # last pack: evacuate PV results (+denominator row)
                        for h, _, _ in sides:
                            nc.vector.tensor_copy(ytAll[:, h, :], psum_y[h][0 : HD + 1, :])
                units.append(u)
                first = False
        return units

    def norm_units(qc):
        ytAll = ytAll_tiles[qc]

        def u(qc=qc, ytAll=ytAll):
            # repartition the 8 denominator rows [1, 8*512] -> [128, 32]
            s4 = r_pool.tile([P, 32], BF16, tag="s4")
            nc.sync.dma_start(s4[:], ytAll[HD : HD + 1, :, :])
            r4 = r_pool.tile([P, 32], F32, tag="r4")
            nc.vector.reciprocal(r4[:], s4[:])
            r4b = r_pool.tile([P, 32], BF16, tag="r4b")
            nc.vector.tensor_copy(r4b[:], r4[:])
            rr = rr_pool.tile([1, NH * 512], BF16, tag="rr")
            nc.sync.dma_start(rr[:], r4b[:])
            for h in range(NH):
                t, row0 = h // 2, (h % 2) * HD
                rfull = rf_pool.tile([HD, 512], BF16, tag="rfull")
                nc.gpsimd.partition_broadcast(rfull[:], rr[0:1, ds(h * 512, 512)])
                nc.vector.tensor_mul(
                    yT[t][row0 : row0 + HD, ts(qc, 512)], ytAll[0:HD, h, :], rfull[:]
                )
        return [u]

    def proj_units(qc):
        units = []
        for st in range(qc * 4, qc * 4 + 4):
            for ncol in range(2):
                def u(st=st, ncol=ncol):
                    po = dense_ps.tile([P, 512], F32, tag="pc", name=f"po{st}{ncol}")
                    for kc in range(N_KCHUNK):
                        nc.tensor.matmul(
                            po[:],
                            lhsT=yT[kc][:, ts(st, P)],
                            rhs=wo[kc][:, ds(ncol * 512, 512)],
                            start=(kc == 0),
                            stop=(kc == N_KCHUNK - 1),
                        )
                    ot = o_pool.tile([P, 512], F32, tag="o")
                    nc.vector.tensor_copy(ot[:], po[:])
                    nc.sync.dma_start(out_ap[ts(st, P), ds(ncol * 512, 512)], ot[:])
                units.append(u)
        return units

    # ---------------- interleaved emission ----------------

    def run_phase(main, fillers):
        if not main:
            for u in fillers:
                u()
            return
        ratio = len(fillers) / len(main)
        acc, fi = 0.0, 0
        for u in main:
            u()
            acc += ratio
            while acc >= 1.0 and fi < len(fillers):
                fillers[fi]()
                fi += 1
                acc -= 1.0
        while fi < len(fillers):
            fillers[fi]()
            fi += 1

    emit_weight_loads()
    for u in transpose_units(0, emit_x_loads(0)):
        u()
    run_phase(qkv_units(0), transpose_units(1, emit_x_loads(1)))
    run_phase(attn_units(0) + norm_units(0), transpose_units(2, emit_x_loads(2)) + qkv_units(1))
    run_phase(attn_units(1) + norm_units(1), transpose_units(3, emit_x_loads(3)) + qkv_units(2))
    run_phase(attn_units(2) + norm_units(2), qkv_units(3))
    run_phase(attn_units(3) + norm_units(3), proj_units(0) + proj_units(1) + proj_units(2))
    run_phase(proj_units(3), [])


def build_nc():
    nc = bacc.Bacc("TRN2", target_bir_lowering=False, debug=False)
    x_ap = nc.dram_tensor("x", [S, D], F32, kind="ExternalInput").ap()
    wq_ap = nc.dram_tensor("wq", [D, DSH], F32, kind="ExternalInput").ap()
    wk_ap = nc.dram_tensor("wk", [D, DSH], F32, kind="ExternalInput").ap()
    wv_ap = nc.dram_tensor("wv", [D, DSH], F32, kind="ExternalInput").ap()
    wo_ap = nc.dram_tensor("wo", [DSH, D], F32, kind="ExternalInput").ap()
    out_ap = nc.dram_tensor("out", [S, D], F32, kind="ExternalOutput").ap()
    with tile.TileContext(nc) as tc:
        with ExitStack() as ctx:
            _emit(ctx, tc, x_ap, wq_ap, wk_ap, wv_ap, wo_ap, out_ap)
    nc.compile()
    return nc


_NC = None


def _get_nc():
    global _NC
    if _NC is None:
        _NC = build_nc()
    return _NC


def make_in_maps(x, Wqkv, Wo):
    Wq, Wk, Wv = Wqkv[:, 0:D], Wqkv[:, D : 2 * D], Wqkv[:, 2 * D : 3 * D]
    in_maps = []
    for c in range(8):
        b, hh = c // 2, c % 2
        cs = slice(hh * DSH, (hh + 1) * DSH)
        in_maps.append(
            {
                "x": np.ascontiguousarray(x[b], dtype=np.float32),
                "wq": np.ascontiguousarray(Wq[:, cs], dtype=np.float32),
                "wk": np.ascontiguousarray(Wk[:, cs], dtype=np.float32),
                "wv": np.ascontiguousarray(Wv[:, cs], dtype=np.float32),
                "wo": np.ascontiguousarray(Wo[cs, :], dtype=np.float32),
            }
        )
    return in_maps


def kernel(x, Wqkv, Wo, trace=False):
    x = np.asarray(x)
    Wqkv = np.asarray(Wqkv)
    Wo = np.asarray(Wo)
    nc = _get_nc()
    res = run_bass_kernel_spmd(nc, make_in_maps(x, Wqkv, Wo), list(range(8)), trace=trace)
    out = np.empty((4, S, D), np.float32)
    for b in range(4):
        out[b] = res.results[2 * b]["out"] + res.results[2 * b + 1]["out"]
    if trace:
        kernel.last_exec_time_ns = res.exec_time_ns
        kernel.last_results = res
    return out
